# revision 9
# baseline (speedup 1.0000x reference)
"""Trainium2 Bass kernel for nn_Encoder_80616536146562 (graph-LSTM encoder).

Reference computation (B=4, T=12, N=4096, F=16):
  per step t:
    gx = relu(adj @ (x_t @ Wx) + bx); gh = relu(adj @ (h @ Wh) + bh)
    LSTM gates -> c, h2
    sh = relu(adj @ (h2 @ Wsh) + bsh); sm = relu(adj @ (m @ Wsm) + bsm)
    memory gates -> m, h = m * o2
  outputs: hidden_states [B,T,N,F], last_h, last_c, last_m

Strategy: 8-way row-shard of adj (512 rows/core) kept resident in SBUF as
fp16 (scaled by 4096 to avoid fp16 subnormals; un-scaled when leaving PSUM).
Associativity: compute Y = adj @ z first (F cols per state), then tiny Y @ W
matmuls on-chip (PE transpose + W-matmuls). adj@x_t is precomputed for all t
upfront. Node states h/m/h2 are all-gathered in fp16 across the 8 cores twice
per step via collective_compute through DRAM bounce buffers.
"""
import sys

if "/opt/trn_rl_repo" not in sys.path:
    sys.path.insert(0, "/opt/trn_rl_repo")

import numpy as np

B, T, N, F = 4, 12, 4096, 16
NCORES = 8
RPC = N // NCORES          # rows per core = 512
NM = RPC // 128            # m-tiles per core = 4
NK = N // 128              # k-tiles = 32
G1 = 4 * F                 # 64  (f|i|c|o gates)
G2 = 3 * F                 # 48  (i|g|o gates)
BF = B * F                 # 64
SCALE = 4096.0

_cache = {}


def _build_nc():
    import concourse.bass as bass
    import concourse.mybir as mybir
    import concourse.tile as tile

    f32, f16 = mybir.dt.float32, mybir.dt.float16
    AF = mybir.ActivationFunctionType
    OP = mybir.AluOpType

    nc = bass.Bass(trn_type="TRN2", num_devices=NCORES)

    # ---- per-core inputs ----
    adjT = nc.dram_tensor("adjT", [N, RPC], f32, kind="ExternalInput")
    xr = nc.dram_tensor("xr", [N, T * BF], f32, kind="ExternalInput")   # x[n, (t,b,f)]
    wx = nc.dram_tensor("wx", [BF, B * G1], f32, kind="ExternalInput")
    wh = nc.dram_tensor("wh", [BF, B * G1], f32, kind="ExternalInput")
    wsh = nc.dram_tensor("wsh", [BF, B * G2], f32, kind="ExternalInput")
    wsm = nc.dram_tensor("wsm", [BF, B * G2], f32, kind="ExternalInput")
    # bias tiles, replicated over m (and padded to 256 per (m,b) for stage2)
    bxt = nc.dram_tensor("bxt", [128, NM * B * G1], f32, kind="ExternalInput")
    bht = nc.dram_tensor("bht", [128, NM * B * G1], f32, kind="ExternalInput")
    bsht = nc.dram_tensor("bsht", [128, NM * B * G1], f32, kind="ExternalInput")
    bsmt = nc.dram_tensor("bsmt", [128, NM * B * G1], f32, kind="ExternalInput")
    ident = nc.dram_tensor("ident", [128, 128], f32, kind="ExternalInput")
    # ---- per-core outputs ----
    hs = nc.dram_tensor("hs", [T, NM, B, 128, F], f32, kind="ExternalOutput")
    lc = nc.dram_tensor("lc", [NM, B, 128, F], f32, kind="ExternalOutput")
    lm = nc.dram_tensor("lm", [NM, B, 128, F], f32, kind="ExternalOutput")

    PBG1 = B * G1          # 256 = per-m gate block (stage1); stage2 padded to same

    with tile.TileContext(nc) as tc:
        with tc.tile_pool(name="const", bufs=1) as constp, \
             tc.tile_pool(name="stage", bufs=2) as stagep, \
             tc.tile_pool(name="state", bufs=1) as statep, \
             tc.tile_pool(name="dram", bufs=2, space="DRAM") as dramp, \
             tc.tile_pool(name="work", bufs=2) as workp:

            # ===== constants / weights =====
            adj16 = constp.tile([128, NK * RPC], f16)        # 4 MB: lhsT tiles
            for k in range(NK):
                a32 = stagep.tile([128, RPC], f32, tag="a32")
                nc.sync.dma_start(a32[:], adjT[k * 128:(k + 1) * 128, :])
                nc.vector.tensor_scalar_mul(
                    adj16[:, k * RPC:(k + 1) * RPC], a32[:], SCALE)

            id16 = constp.tile([128, 128], f16)
            i32 = stagep.tile([128, 128], f32, tag="a32")
            nc.sync.dma_start(i32[:], ident[:])
            nc.vector.tensor_copy(id16[:], i32[:])

            w16 = {}
            for name, t_, gw in (("wx", wx, B * G1), ("wh", wh, B * G1),
                                 ("wsh", wsh, B * G2), ("wsm", wsm, B * G2)):
                ww = stagep.tile([BF, gw], f32, tag="w32", name=f"w32_{name}")
                nc.sync.dma_start(ww[:], t_[:])
                w1 = constp.tile([BF, gw], f16, name=f"w16_{name}")
                nc.vector.tensor_copy(w1[:], ww[:])
                w16[name] = w1

            bias = {}
            for name, t_ in (("bxt", bxt), ("bht", bht),
                             ("bsht", bsht), ("bsmt", bsmt)):
                bb = constp.tile([128, NM * B * G1], f32, name=f"b_{name}")
                nc.sync.dma_start(bb[:], t_[:])
                bias[name] = bb

            # ===== x-pass: Ax[m] = (adj @ x)  for all t, fp16, un-scaled =====
            Ax = [constp.tile([128, T * BF], f16, name=f"Ax{m}") for m in range(NM)]
            HALF = T * BF // 2                                # 384 cols per psum
            with tc.tile_pool(name="xps", bufs=1, space="PSUM") as xps:
                ax_ps = [[xps.tile([128, HALF], f32, name=f"axps{m}_{h}")
                          for h in range(2)] for m in range(NM)]
                for k in range(NK):
                    xs32 = stagep.tile([128, T * BF], f32, tag="xs32")
                    nc.sync.dma_start(xs32[:], xr[k * 128:(k + 1) * 128, :])
                    x16k = stagep.tile([128, T * BF], f16, tag="x16k")
                    nc.vector.tensor_copy(x16k[:], xs32[:])
                    for m in range(NM):
                        for h in range(2):
                            nc.tensor.matmul(
                                ax_ps[m][h][:],
                                adj16[:, k * RPC + m * 128:k * RPC + (m + 1) * 128],
                                x16k[:, h * HALF:(h + 1) * HALF],
                                start=(k == 0), stop=(k == NK - 1))
                for m in range(NM):
                    for h in range(2):
                        nc.vector.tensor_scalar_mul(
                            Ax[m][:, h * HALF:(h + 1) * HALF],
                            ax_ps[m][h][:], 1.0 / SCALE)

            # AxT[(b,f), (t, m, node)] fp16 — transposed x-conv activations
            AxT = constp.tile([BF, T * NM * 128], f16)
            with tc.tile_pool(name="tps", bufs=4, space="PSUM") as tps:
                for t in range(T):
                    for m in range(NM):
                        tp = tps.tile([BF, 128], f16, tag="tp")
                        nc.tensor.transpose(
                            tp[:], Ax[m][:, t * BF:(t + 1) * BF], id16[:])
                        nc.vector.tensor_copy(
                            AxT[:, (t * NM + m) * 128:(t * NM + m + 1) * 128], tp[:])

            # ===== precompute rgx = relu(Ax @ Wx + bx) for all t =====
            gxr = constp.tile([128, T * NM * PBG1], f32)     # 48KB/partition
            with tc.tile_pool(name="gxps", bufs=2, space="PSUM") as gxps:
                for t in range(T):
                    gxa = gxps.tile([128, NM, PBG1], f32, tag="gxa")
                    for m in range(NM):
                        nc.tensor.matmul(
                            gxa[:, m, :],
                            AxT[:, (t * NM + m) * 128:(t * NM + m + 1) * 128],
                            w16["wx"][:], start=True, stop=True)
                    gsl = gxr[:, t * NM * PBG1:(t + 1) * NM * PBG1]
                    nc.vector.tensor_tensor(gsl, gxa[:].rearrange("p m g -> p (m g)"),
                                            bias["bxt"][:], OP.add)
                    nc.scalar.activation(gsl, gsl, AF.Relu)

            # ===== recurrent state =====
            # gathered state, chunked by m-pair: chunk c holds global k-tiles
            # 4r+2c+k2 at column block (r*2+k2)
            h2c = [statep.tile([128, 16 * BF], f16, name=f"h2c{c}")
                   for c in range(2)]
            hmc = [statep.tile([128, 16 * 2 * BF], f16, name=f"hmc{c}")
                   for c in range(2)]
            c32 = statep.tile([128, NM * BF], f32)
            m32 = statep.tile([128, NM * BF], f32)
            nc.gpsimd.memset(hmc[0][:], 1.0)
            nc.gpsimd.memset(hmc[1][:], 1.0)
            nc.gpsimd.memset(c32[:], 1.0)
            nc.gpsimd.memset(m32[:], 1.0)

            NG2 = B * G2                   # 192
            # k order: chunk0's tiles first, then chunk1's
            KORD = [(c, r, k2) for c in range(2) for r in range(NCORES)
                    for k2 in range(2)]

            with tc.tile_pool(name="py1", bufs=1, space="PSUM") as py1, \
                 tc.tile_pool(name="pt1", bufs=3, space="PSUM") as pt1, \
                 tc.tile_pool(name="pg", bufs=2, space="PSUM") as pgp:

                for t in range(T):
                    # ---------- stage 1: Y1 = adj @ [h|m] ----------
                    y1 = py1.tile([128, NM, 2 * BF], f32, tag="y1")
                    for m in range(NM):
                        for i, (c, r, k2) in enumerate(KORD):
                            k = 4 * r + 2 * c + k2
                            nc.tensor.matmul(
                                y1[:, m, :],
                                adj16[:, k * RPC + m * 128:k * RPC + (m + 1) * 128],
                                hmc[c][:, (r * 2 + k2) * 2 * BF:
                                       (r * 2 + k2 + 1) * 2 * BF],
                                start=(i == 0), stop=(i == NK - 1))
                    y1s = workp.tile([128, NM, 2 * BF], f16, tag="y1s")
                    nc.vector.tensor_scalar_mul(y1s[:], y1[:], 1.0 / SCALE)
                    t1h_ps = pt1.tile([BF, NM * 128], f16, tag="sm1")
                    t1m_ps = pt1.tile([BF, NM * 128], f16, tag="sm1", name="t1mps")
                    for m in range(NM):
                        nc.tensor.transpose(
                            t1h_ps[:, m * 128:(m + 1) * 128],
                            y1s[:, m, 0:BF], id16[:])
                        nc.tensor.transpose(
                            t1m_ps[:, m * 128:(m + 1) * 128],
                            y1s[:, m, BF:2 * BF], id16[:])
                    t1h = workp.tile([BF, NM * 128], f16, tag="t1h")
                    nc.vector.tensor_copy(t1h[:], t1h_ps[:])
                    t1m = workp.tile([BF, NM * 128], f16, tag="t1m")
                    nc.vector.tensor_copy(t1m[:], t1m_ps[:])

                    gh = pgp.tile([128, NM, PBG1], f32, tag="g")
                    for m in range(NM):
                        nc.tensor.matmul(
                            gh[:, m, :], t1h[:, m * 128:(m + 1) * 128],
                            w16["wh"][:], start=True, stop=True)
                    # s = relu(gx+bx) [precomputed] + relu(gh+bh), gate-major:
                    # per m: [f(64) | i(64) | c(64) | o(64)] with (b,f) inner
                    pre = workp.tile([128, NM, PBG1], f32, tag="pre", bufs=1)
                    nc.vector.tensor_tensor(
                        pre[:].rearrange("p m g -> p (m g)"),
                        gh[:].rearrange("p m g -> p (m g)"),
                        bias["bht"][:], OP.add)
                    s = workp.tile([128, NM, PBG1], f32, tag="s", bufs=1)
                    nc.vector.scalar_tensor_tensor(
                        s[:].rearrange("p m g -> p (m g)"),
                        pre[:].rearrange("p m g -> p (m g)"), 0.0,
                        gxr[:, t * NM * PBG1:(t + 1) * NM * PBG1],
                        OP.max, OP.add)
                    # sigmoid(f,i) ; tanh(c) ; sigmoid(o)
                    nc.scalar.activation(s[:, :, 0:2 * BF], s[:, :, 0:2 * BF],
                                         AF.Sigmoid)
                    nc.scalar.activation(s[:, :, 3 * BF:4 * BF],
                                         s[:, :, 3 * BF:4 * BF], AF.Sigmoid)
                    nc.scalar.activation(s[:, :, 2 * BF:3 * BF],
                                         s[:, :, 2 * BF:3 * BF], AF.Tanh)
                    cv = c32[:].rearrange("p (m c) -> p m c", m=NM)
                    tmp1 = workp.tile([128, NM, BF], f32, tag="tmp1")
                    tmp2 = workp.tile([128, NM, BF], f32, tag="tmp2")
                    nc.vector.tensor_tensor(tmp1[:], s[:, :, 0:BF], cv, OP.mult)
                    nc.vector.tensor_tensor(tmp2[:], s[:, :, BF:2 * BF],
                                            s[:, :, 2 * BF:3 * BF], OP.mult)
                    nc.vector.tensor_tensor(cv, tmp1[:], tmp2[:], OP.add)
                    th = workp.tile([128, NM * BF], f32, tag="th")
                    nc.scalar.activation(th[:], c32[:], AF.Tanh)
                    h2_16 = workp.tile([128, NM, BF], f16, tag="h2_16")
                    nc.vector.tensor_tensor(
                        h2_16[:], s[:, :, 3 * BF:4 * BF],
                        th[:].rearrange("p (m c) -> p m c", m=NM), OP.mult)

                    # ---------- all-gather h2 (2 chunks of 2 m-tiles) ----------
                    agA_out = []
                    for c in range(2):
                        ain = dramp.tile([2 * 128, BF], f16, tag=f"agA_in{c}",
                                         name=f"agAin{c}")
                        nc.sync.dma_start(
                            ain.rearrange("(m p) f -> p m f", p=128),
                            h2_16[:, 2 * c:2 * c + 2, :])
                        aout = dramp.tile([16 * 128, BF], f16, tag=f"agA_out{c}",
                                          name=f"agAout{c}", addr_space="Shared")
                        nc.gpsimd.collective_compute(
                            "AllGather", OP.bypass,
                            replica_groups=[list(range(NCORES))],
                            ins=[ain[:]], outs=[aout[:]])
                        agA_out.append(aout)

                    # sm matmul only needs t1m — runs during the gather
                    sm = pgp.tile([128, NM, PBG1], f32, tag="g", name="sm")
                    for m in range(NM):
                        nc.tensor.matmul(
                            sm[:, m, 0:NG2], t1m[:, m * 128:(m + 1) * 128],
                            w16["wsm"][:], start=True, stop=True)
                    rsm = workp.tile([128, NM, PBG1], f32, tag="rsm", bufs=1)
                    nc.vector.tensor_tensor(
                        rsm[:, :, 0:NG2], sm[:, :, 0:NG2],
                        bias["bsmt"][:].rearrange("p (m g) -> p m g", m=NM)
                        [:, :, 0:NG2], OP.add)

                    for c in range(2):
                        nc.sync.dma_start(
                            h2c[c][:].rearrange("p (j f) -> p j f", j=16),
                            agA_out[c].rearrange("(j p) f -> p j f", p=128))

                    # ---------- stage 2: Y2 = adj @ h2 ----------
                    y2 = pt1.tile([128, NM, BF], f32, tag="sm1", name="y2")
                    for m in range(NM):
                        for i, (c, r, k2) in enumerate(KORD):
                            k = 4 * r + 2 * c + k2
                            nc.tensor.matmul(
                                y2[:, m, :],
                                adj16[:, k * RPC + m * 128:k * RPC + (m + 1) * 128],
                                h2c[c][:, (r * 2 + k2) * BF:(r * 2 + k2 + 1) * BF],
                                start=(i == 0), stop=(i == NK - 1))
                    y2s = workp.tile([128, NM, BF], f16, tag="y2s")
                    nc.vector.tensor_scalar_mul(y2s[:], y2[:], 1.0 / SCALE)
                    t2_ps = pt1.tile([BF, NM * 128], f16, tag="sm1", name="t2ps")
                    for m in range(NM):
                        nc.tensor.transpose(
                            t2_ps[:, m * 128:(m + 1) * 128], y2s[:, m, :], id16[:])
                    t2 = workp.tile([BF, NM * 128], f16, tag="t2")
                    nc.vector.tensor_copy(t2[:], t2_ps[:])

                    sh = pgp.tile([128, NM, PBG1], f32, tag="g", name="sh")
                    for m in range(NM):
                        nc.tensor.matmul(
                            sh[:, m, 0:NG2], t2[:, m * 128:(m + 1) * 128],
                            w16["wsh"][:], start=True, stop=True)
                    # s2 = sigmoid(relu(sh+bsh) + relu(sm+bsm)), gate-major
                    # per m: [i(64) | g(64) | o(64)] in first 192 of 256
                    pre2 = workp.tile([128, NM, PBG1], f32, tag="pre2", bufs=1)
                    nc.vector.tensor_tensor(
                        pre2[:, :, 0:NG2], sh[:, :, 0:NG2],
                        bias["bsht"][:].rearrange("p (m g) -> p m g", m=NM)
                        [:, :, 0:NG2], OP.add)
                    s2 = workp.tile([128, NM, PBG1], f32, tag="s2", bufs=1)
                    nc.vector.tensor_scalar(rsm[:, :, 0:NG2], rsm[:, :, 0:NG2],
                                            0.0, None, OP.max)
                    nc.vector.scalar_tensor_tensor(
                        s2[:, :, 0:NG2], pre2[:, :, 0:NG2], 0.0,
                        rsm[:, :, 0:NG2], OP.max, OP.add)
                    nc.scalar.activation(s2[:, :, 0:NG2], s2[:, :, 0:NG2],
                                         AF.Sigmoid)

                    mv = m32[:].rearrange("p (m c) -> p m c", m=NM)
                    tmp1 = workp.tile([128, NM, BF], f32, tag="tmp1")
                    tmp2 = workp.tile([128, NM, BF], f32, tag="tmp2")
                    # m = i2*m + (1-i2)*g2 ; h = m*o2
                    nc.vector.tensor_tensor(tmp1[:], s2[:, :, 0:BF], mv, OP.mult)
                    nc.vector.tensor_tensor(tmp2[:], s2[:, :, 0:BF],
                                            s2[:, :, BF:2 * BF], OP.mult)
                    nc.vector.tensor_tensor(tmp2[:], s2[:, :, BF:2 * BF],
                                            tmp2[:], OP.subtract)
                    nc.vector.tensor_tensor(mv, tmp1[:], tmp2[:], OP.add)
                    hnew32 = workp.tile([128, NM * BF], f32, tag="hnew32")
                    nc.vector.tensor_tensor(
                        hnew32[:].rearrange("p (m c) -> p m c", m=NM),
                        mv, s2[:, :, 2 * BF:3 * BF], OP.mult)
                    hmn16 = workp.tile([128, NM, 2 * BF], f16, tag="hmn16")
                    nc.vector.tensor_copy(
                        hmn16[:, :, 0:BF],
                        hnew32[:].rearrange("p (m c) -> p m c", m=NM))
                    nc.vector.tensor_copy(
                        hmn16[:, :, BF:2 * BF],
                        m32[:].rearrange("p (m c) -> p m c", m=NM))

                    # ---------- all-gather [h|m] (2 chunks) + outputs ----------
                    if t < T - 1:
                        for c in range(2):
                            bin_ = dramp.tile([2 * 128, 2 * BF], f16,
                                              tag=f"agB_in{c}", name=f"agBin{c}")
                            nc.sync.dma_start(
                                bin_.rearrange("(m p) g -> p m g", p=128),
                                hmn16[:, 2 * c:2 * c + 2, :])
                            bout = dramp.tile([16 * 128, 2 * BF], f16,
                                              tag=f"agB_out{c}", name=f"agBout{c}",
                                              addr_space="Shared")
                            nc.gpsimd.collective_compute(
                                "AllGather", OP.bypass,
                                replica_groups=[list(range(NCORES))],
                                ins=[bin_[:]], outs=[bout[:]])
                            nc.sync.dma_start(
                                hmc[c][:].rearrange("p (j g) -> p j g", j=16),
                                bout.rearrange("(j p) g -> p j g", p=128))

                    nc.sync.dma_start(
                        hs[t].rearrange("m b p f -> p m b f"),
                        hnew32[:].rearrange("p (m b f) -> p m b f", m=NM, b=B))

                nc.sync.dma_start(
                    lc[:].rearrange("m b p f -> p m b f"),
                    c32[:].rearrange("p (m b f) -> p m b f", m=NM, b=B))
                nc.sync.dma_start(
                    lm[:].rearrange("m b p f -> p m b f"),
                    m32[:].rearrange("p (m b f) -> p m b f", m=NM, b=B))

    _legalize_waits(nc)
    return nc


def _legalize_waits(nc):
    """Walrus accepts at most 1 sync-wait per instruction (2 for
    EventSemaphore). Move excess waits onto standalone EventSemaphore
    instructions on the same engine, inserted just before."""
    import concourse.mybir as mybir

    n_split = 0
    for fn in nc.m.functions:
        for bb in fn.blocks:
            newl = []
            changed = False
            for ins in bb.instructions:
                si = ins.sync_info
                waits = list(si.on_wait) if (si is not None and si.on_wait) else []
                cap = 2 if isinstance(ins, mybir.InstEventSemaphore) else 1
                if len(waits) > cap:
                    extra, keep = waits[:-cap], waits[-cap:]
                    for i in range(0, len(extra), 2):
                        ev = mybir.InstEventSemaphore(
                            name=f"{ins.name}_xw{i}",
                            engine=ins.engine,
                            sync_info=mybir.SyncInfo(
                                on_wait=list(extra[i:i + 2]), on_update=[]),
                        )
                        newl.append(ev)
                        n_split += 1
                    ins.sync_info = mybir.SyncInfo(
                        on_wait=list(keep), on_update=list(si.on_update))
                    changed = True
                newl.append(ins)
            if changed:
                bb.instructions = newl
    return n_split


def run(inputs, trace=False):
    from concourse.bass_utils import run_bass_kernel_spmd

    if "nc" not in _cache:
        _cache["nc"] = _build_nc()
    nc = _cache["nc"]

    x = np.ascontiguousarray(inputs["x"], dtype=np.float32)
    adj = np.ascontiguousarray(inputs["adj"], dtype=np.float32)
    # x rearranged to [n, (t, b, f)]
    xr = np.ascontiguousarray(x.transpose(2, 1, 0, 3)).reshape(N, T * B * F)
    ident = np.eye(128, dtype=np.float32)

    def btile1(bvec):
        v = np.tile(np.tile(np.asarray(bvec, np.float32), B), NM)   # [NM*B*G1]
        return np.ascontiguousarray(np.broadcast_to(v, (128, v.size)))

    def btile2(bvec):
        v = np.zeros((NM, B, G1), np.float32)
        bv = np.asarray(bvec, np.float32).reshape(B if False else 1, -1)
        # layout per m: (b, g2) packed contiguously in first B*G2 of B*G1
        flat = np.tile(np.asarray(bvec, np.float32), B)              # [B*G2]
        v2 = np.zeros((NM, B * G1), np.float32)
        v2[:, 0:B * G2] = flat
        v2 = v2.reshape(-1)
        return np.ascontiguousarray(np.broadcast_to(v2, (128, v2.size)))

    def bdiag_gm(w):
        # block-diagonal over batch, columns permuted gate-major:
        # out[b*F+fi, g*B*F + b*F + fo] = w[fi, g*F+fo]
        w = np.asarray(w, np.float32)
        f, gtot = w.shape
        ng = gtot // F
        out = np.zeros((B * f, ng * B * F), np.float32)
        for b in range(B):
            for g in range(ng):
                out[b * f:(b + 1) * f, g * B * F + b * F:g * B * F + (b + 1) * F] = \
                    w[:, g * F:(g + 1) * F]
        return out

    def btile_gm(bvec, ng):
        # [128, NM * ng * B * F] with per-m layout (gate, b, f), padded to
        # NM * B * G1 total
        arr = np.asarray(bvec, np.float32).reshape(ng, F)
        v = np.broadcast_to(arr[:, None, :], (ng, B, F)).reshape(-1)  # [ng*B*F]
        blk = np.zeros(B * G1, np.float32)
        blk[:v.size] = v
        full = np.tile(blk, NM)
        return np.ascontiguousarray(np.broadcast_to(full, (128, full.size)))

    common = {
        "xr": xr,
        "wx": bdiag_gm(inputs["Wx"]),
        "wh": bdiag_gm(inputs["Wh"]),
        "wsh": bdiag_gm(inputs["Wsh"]),
        "wsm": bdiag_gm(inputs["Wsm"]),
        "bxt": btile_gm(inputs["bx"], 4),
        "bht": btile_gm(inputs["bh"], 4),
        "bsht": btile_gm(inputs["bsh"], 3),
        "bsmt": btile_gm(inputs["bsm"], 3),
        "ident": ident,
    }
    in_maps = []
    for c in range(NCORES):
        m = dict(common)
        m["adjT"] = np.ascontiguousarray(adj[c * RPC:(c + 1) * RPC, :].T)
        in_maps.append(m)

    res = run_bass_kernel_spmd(
        nc, in_maps, core_ids=list(range(NCORES)), trace=trace)

    hs_parts, lc_parts, lm_parts = [], [], []
    for c in range(NCORES):
        r = res.results[c]
        # hs [T, NM, B, 128, F] -> [B, T, RPC, F]
        hs_parts.append(r["hs"].transpose(2, 0, 1, 3, 4).reshape(B, T, RPC, F))
        lc_parts.append(r["lc"].transpose(1, 0, 2, 3).reshape(B, RPC, F))
        lm_parts.append(r["lm"].transpose(1, 0, 2, 3).reshape(B, RPC, F))
    hidden = np.concatenate(hs_parts, axis=2)
    last_c = np.concatenate(lc_parts, axis=1)
    last_m = np.concatenate(lm_parts, axis=1)
    last_h = np.ascontiguousarray(hidden[:, T - 1])
    return (hidden, last_h, last_c, last_m), res


def kernel(**inputs):
    out, _ = run(inputs, trace=False)
    return out
